# revision 13
# baseline (speedup 1.0000x reference)
"""Block-sparse attention (block-diagonal mask, full-row softmax) on 8 trn2 cores.

Reference semantics (B=1, H=16, S=4096, D=64, BLOCK=64):
    scores  = (Q @ K^T) / 8                     [S, S] per head
    scores *= blockdiag_mask                    (off-block -> 0, NOT -inf)
    weights = softmax(scores, axis=-1)          (over the FULL row)
    out     = weights @ V

Off-block entries contribute exp(0)=1 to the softmax, so for row q in
block b:
    num_q   = sum_{k in b} e_qk v_k - V_bsum(b) + V_total
    denom_q = sum_{k in b} e_qk - 64 + S
    out_q   = num_q / denom_q
Only the diagonal 64x64 blocks are ever materialized.

Sharding: 16 heads over 8 cores -> 2 heads/core, no cross-core comms.

Per-core pipeline (per 128-row chunk = 2 blocks):
  - Q/K are DMA-cast to bf16 on load; PE-transposed to QT/KT [64(d), 128(s)].
  - One 66-partition bf16 matmul produces S^T[k, q] for the whole chunk;
    two constant "mask rows" add -M^2 to every cross-block element so a
    single full-tile exp yields exact 0 off-block.
  - E^T @ [V|1] (bf16), a constant block-diagonal -1 matmul (the "-bsum"
    term), and a rank-1 [Vtot|S] update accumulate num|denom in PSUM fp32.
  - reciprocal + per-partition scale produce the output chunk.
"""

import numpy as np

H, S, D = 16, 4096, 64
HPC = 2  # heads per core
NCORES = 8
CHUNK = 128
NCHUNK = S // CHUNK  # 32
SCALE = 0.125  # 1/sqrt(D)
MASK_M = 64.0  # mask-row magnitude: M^2 * SCALE = 512 >> exp underflow

_CACHE = {}


def _build_bass():
    import concourse.bass as bass
    import concourse.bacc as bacc
    import concourse.tile as tile
    from concourse import mybir
    from concourse.masks import make_identity

    f32 = mybir.dt.float32
    bf16 = mybir.dt.bfloat16
    EXP = mybir.ActivationFunctionType.Exp
    COPY = mybir.ActivationFunctionType.Copy

    nc = bacc.Bacc(
        "TRN2", target_bir_lowering=False, debug=False, num_devices=NCORES
    )
    q_d = nc.dram_tensor("query", [HPC, S, D], f32, kind="ExternalInput")
    k_d = nc.dram_tensor("key", [HPC, S, D], f32, kind="ExternalInput")
    v_d = nc.dram_tensor("value", [HPC, S, D], f32, kind="ExternalInput")
    o_d = nc.dram_tensor("out", [HPC, S, D], f32, kind="ExternalOutput")

    NT = 3  # rotation depth for the fixed transpose-staging tiles

    with tile.TileContext(nc) as tc:
        with (
            tc.tile_pool(name="consts", bufs=1) as consts,
            tc.tile_pool(name="heads", bufs=2) as heads,
            tc.tile_pool(name="work", bufs=3) as work,
            tc.tile_pool(name="vt", bufs=2) as vtp,
            tc.tile_pool(name="ps_t", bufs=2, space="PSUM") as ps_t,
            tc.tile_pool(name="ps_s", bufs=2, space="PSUM") as ps_s,
            tc.tile_pool(name="ps_o", bufs=2, space="PSUM") as ps_o,
            tc.tile_pool(name="ps_vt", bufs=1, space="PSUM") as ps_vt,
        ):
            identb = consts.tile([128, 128], bf16, tag="identb")
            make_identity(nc, identb)
            ones_row = consts.tile([1, 128], bf16, tag="ones_row")
            nc.gpsimd.memset(ones_row, 1.0)
            ones_col = consts.tile([128, 1], f32, tag="ones_col")
            nc.gpsimd.memset(ones_col, 1.0)

            # Block-diagonal -1 (the "-bsum" correction as a matmul weight)
            negblk = consts.tile([128, 128], bf16, tag="negblk")
            nc.gpsimd.memset(negblk, 0.0)
            nc.gpsimd.memset(negblk[0:64, 0:64], -1.0)
            nc.gpsimd.memset(negblk[64:128, 64:128], -1.0)

            # Fixed transpose-staging tiles [66, 256]: rows 0:64 = QT | KT,
            # rows 64:66 = scores mask rows (written once):
            #   col layout: [QT q 0:128 | KT k 128:256]
            #   row 64:  q-side 0 / -M   k-side  M / 0
            #   row 65:  q-side -M / 0   k-side  0 / M
            tsbs = []
            for i in range(NT):
                t = consts.tile([66, 256], bf16, tag=f"tsb{i}")
                nc.gpsimd.memset(t[64:66, :], 0.0)
                # q-side mask rows (cols 0:128, viewed [2(r), 2(jb), 64]):
                # -M where r + jb == 1
                nc.gpsimd.affine_select(
                    out=t[64:66, 0:128].rearrange("p (b j) -> p b j", b=2),
                    in_=t[64:66, 0:128].rearrange("p (b j) -> p b j", b=2),
                    compare_op=mybir.AluOpType.not_equal,
                    fill=-MASK_M,
                    base=-1,
                    pattern=[[1, 2], [0, 64]],
                    channel_multiplier=1,
                )
                # k-side mask rows (cols 128:256): +M where r == jb
                nc.gpsimd.affine_select(
                    out=t[64:66, 128:256].rearrange("p (b j) -> p b j", b=2),
                    in_=t[64:66, 128:256].rearrange("p (b j) -> p b j", b=2),
                    compare_op=mybir.AluOpType.not_equal,
                    fill=MASK_M,
                    base=0,
                    pattern=[[-1, 2], [0, 64]],
                    channel_multiplier=1,
                )
                tsbs.append(t)

            for h in range(HPC):
                qh = heads.tile([128, NCHUNK, D], f32, tag="qh")
                kh = heads.tile([128, NCHUNK, D], f32, tag="kh")
                vh = heads.tile([128, NCHUNK, D + 1], f32, tag="vh")
                oh = heads.tile([128, NCHUNK, D], f32, tag="oh")
                nc.sync.dma_start(
                    out=qh, in_=q_d[h].rearrange("(c p) d -> p c d", p=128)
                )
                nc.sync.dma_start(
                    out=kh, in_=k_d[h].rearrange("(c p) d -> p c d", p=128)
                )
                nc.sync.dma_start(
                    out=vh[:, :, 0:D], in_=v_d[h].rearrange("(c p) d -> p c d", p=128)
                )
                nc.vector.memset(vh[:, :, D : D + 1], 1.0)

                # bf16 working copies (GpSimd is otherwise idle)
                qhb = heads.tile([128, NCHUNK, D], bf16, tag="qhb")
                khb = heads.tile([128, NCHUNK, D], bf16, tag="khb")
                vhb = heads.tile([128, NCHUNK, D + 1], bf16, tag="vhb")
                nc.gpsimd.tensor_copy(out=qhb, in_=qh)
                nc.gpsimd.tensor_copy(out=khb, in_=kh)
                nc.gpsimd.tensor_copy(out=vhb, in_=vh)

                # V_total colsum -> vtxb [1, D+1] bf16; element D = S exactly.
                vacc = vtp.tile([128, D + 1], f32, tag="vacc")
                nc.vector.reduce_sum(
                    out=vacc,
                    in_=vh.rearrange("p c d -> p d c"),
                    axis=mybir.AxisListType.X,
                )
                vt_ps = ps_vt.tile([1, D + 1], f32, tag="vt_ps")
                nc.tensor.matmul(vt_ps, ones_col, vacc, start=True, stop=True)
                vtxb = vtp.tile([1, D + 1], bf16, tag="vtxb")
                nc.scalar.copy(out=vtxb, in_=vt_ps)

                for c in range(NCHUNK):
                    # -- QT/KT [64(d), 128(s)] via PE transpose (bf16) --
                    pt = ps_t.tile([64, 256], bf16, tag="pt")
                    nc.tensor.transpose(pt[:, 0:128], qhb[:, c, :], identb)
                    nc.tensor.transpose(pt[:, 128:256], khb[:, c, :], identb)
                    tsb = tsbs[c % NT]
                    nc.vector.tensor_copy(out=tsb[0:64, :], in_=pt)

                    # -- scores S^T[k, q], full chunk, mask rows -> cross=-M^2
                    ps = ps_s.tile([128, 128], f32, tag="ps")
                    nc.tensor.matmul(
                        ps, tsb[:, 128:256], tsb[:, 0:128], start=True, stop=True
                    )

                    # -- E^T = exp(S^T/8): exact 0 on cross-block quadrants --
                    et = work.tile([128, 128], bf16, tag="et")
                    nc.scalar.activation(out=et, in_=ps, func=EXP, scale=SCALE)

                    # -- num|denom = E^T.T @ [V|1] - bsum + [Vtot|S] --
                    po = ps_o.tile([128, D + 1], f32, tag="po")
                    nc.tensor.matmul(
                        po, et, vhb[:, c, :], start=True, stop=False
                    )
                    nc.tensor.matmul(
                        po, negblk, vhb[:, c, :], start=False, stop=False
                    )
                    nc.tensor.matmul(po, ones_row, vtxb, start=False, stop=True)

                    # -- out = num * (1/denom) --
                    rcp = work.tile([128, 1], f32, tag="rcp")
                    nc.vector.reciprocal(out=rcp, in_=po[:, D : D + 1])
                    nc.scalar.activation(
                        out=oh[:, c, :], in_=po[:, 0:D], func=COPY, scale=rcp
                    )

                nc.sync.dma_start(
                    out=o_d[h].rearrange("(c p) d -> p c d", p=128), in_=oh
                )

    nc.compile()
    return nc


def _get_compiled():
    if "nc" not in _CACHE:
        _CACHE["nc"] = _build_bass()
    return _CACHE["nc"]


def make_in_maps(query, key, value):
    q = np.ascontiguousarray(np.asarray(query).reshape(H, S, D), dtype=np.float32)
    k = np.ascontiguousarray(np.asarray(key).reshape(H, S, D), dtype=np.float32)
    v = np.ascontiguousarray(np.asarray(value).reshape(H, S, D), dtype=np.float32)
    in_maps = []
    for i in range(NCORES):
        sl = slice(i * HPC, (i + 1) * HPC)
        in_maps.append(
            {
                "query": np.ascontiguousarray(q[sl]),
                "key": np.ascontiguousarray(k[sl]),
                "value": np.ascontiguousarray(v[sl]),
            }
        )
    return in_maps


def run_spmd(in_maps, **kwargs):
    from concourse.bass_utils import run_bass_kernel_spmd

    nc = _get_compiled()
    return run_bass_kernel_spmd(nc, in_maps, core_ids=list(range(NCORES)), **kwargs)


def assemble(res):
    outs = [res.results[i]["out"] for i in range(NCORES)]
    return np.concatenate(outs, axis=0).reshape(1, H, S, D).astype(np.float32)


def kernel(query: np.ndarray, key: np.ndarray, value: np.ndarray) -> np.ndarray:
    return assemble(run_spmd(make_in_maps(query, key, value)))


# revision 24
# speedup vs baseline: 1.0792x; 1.0792x over previous
"""Block-sparse attention (block-diagonal mask, full-row softmax) on 8 trn2 cores.

Reference semantics (B=1, H=16, S=4096, D=64, BLOCK=64):
    scores  = (Q @ K^T) / 8                     [S, S] per head
    scores *= blockdiag_mask                    (off-block -> 0, NOT -inf)
    weights = softmax(scores, axis=-1)          (over the FULL row)
    out     = weights @ V

Off-block entries contribute exp(0)=1 to the softmax, so for row q in
block b:
    num_q   = sum_{k in b} e_qk v_k - V_bsum(b) + V_total
    denom_q = sum_{k in b} e_qk - 64 + S
    out_q   = num_q / denom_q
Only the diagonal 64x64 blocks are ever materialized.

Sharding: 16 heads over 8 cores -> 2 heads/core, no cross-core comms.

Per-core pipeline, one iteration = a PAIR of 128-row chunks (256 rows):
  - One PE transpose per tensor turns the fp32 [128, 2, 64] pair-slice of
    Q/K into bf16 [128(cc*64+d), 128(s)] -- chunk c on partitions 0:64,
    chunk c+1 on 64:128.
  - Two bf16 matmuls produce S^T[k, q] per chunk into one PSUM tile
    [128, 256] (fp32).
  - One exp over the whole tile; GpSimd zeroes the cross-block quadrants.
  - Per chunk: E^T @ [V|1] (bf16), a constant block-diagonal -1 matmul
    (the "-bsum" term), and a rank-1 [Vtot|S] accumulate num|denom into a
    shared [128, 130] PSUM tile.
  - One reciprocal [128, 2] + one broadcast multiply writes both chunks.
"""

import numpy as np

H, S, D = 16, 4096, 64
HPC = 2  # heads per core
NCORES = 8
CHUNK = 128
NCHUNK = S // CHUNK  # 32
NPAIR = NCHUNK // 2  # 16
SCALE = 0.125  # 1/sqrt(D)

_CACHE = {}


def _build_bass():
    import concourse.bass as bass
    import concourse.bacc as bacc
    import concourse.tile as tile
    from concourse import mybir
    from concourse.masks import make_identity

    f32 = mybir.dt.float32
    bf16 = mybir.dt.bfloat16
    EXP = mybir.ActivationFunctionType.Exp

    nc = bacc.Bacc(
        "TRN2", target_bir_lowering=False, debug=False, num_devices=NCORES
    )
    q_d = nc.dram_tensor("query", [HPC, S, D], f32, kind="ExternalInput")
    k_d = nc.dram_tensor("key", [HPC, S, D], f32, kind="ExternalInput")
    v_d = nc.dram_tensor("value", [HPC, S, D], f32, kind="ExternalInput")
    o_d = nc.dram_tensor("out", [HPC, S, D], f32, kind="ExternalOutput")

    with tile.TileContext(nc) as tc:
        with (
            tc.tile_pool(name="consts", bufs=1) as consts,
            tc.tile_pool(name="heads", bufs=2) as heads,
            tc.tile_pool(name="work", bufs=3) as work,
            tc.tile_pool(name="vt", bufs=2) as vtp,
            tc.tile_pool(name="ps_t", bufs=2, space="PSUM") as ps_t,
            tc.tile_pool(name="ps_s", bufs=2, space="PSUM") as ps_s,
            tc.tile_pool(name="ps_o", bufs=3, space="PSUM") as ps_o,
        ):
            ident = consts.tile([128, 128], f32, tag="ident")
            make_identity(nc, ident)
            ones_row = consts.tile([1, 128], bf16, tag="ones_row")
            nc.gpsimd.memset(ones_row, 1.0)
            ones_colb = consts.tile([128, 1], bf16, tag="ones_colb")
            nc.gpsimd.memset(ones_colb, 1.0)

            # Block-diagonal -1 (the "-bsum" correction as a matmul weight)
            negblk = consts.tile([128, 128], bf16, tag="negblk")
            nc.gpsimd.memset(negblk, 0.0)
            nc.gpsimd.memset(negblk[0:64, 0:64], -1.0)
            nc.gpsimd.memset(negblk[64:128, 64:128], -1.0)

            for h in range(HPC):
                qh = heads.tile([128, NCHUNK, D], f32, tag="qh")
                kh = heads.tile([128, NCHUNK, D], f32, tag="kh")
                vh = heads.tile([128, NCHUNK, D + 1], f32, tag="vh")
                oh = heads.tile([128, NCHUNK, D], f32, tag="oh")
                nc.sync.dma_start(
                    out=vh[:, :, 0:D], in_=v_d[h].rearrange("(c p) d -> p c d", p=128)
                )
                nc.sync.dma_start(
                    out=qh, in_=q_d[h].rearrange("(c p) d -> p c d", p=128)
                )
                nc.sync.dma_start(
                    out=kh, in_=k_d[h].rearrange("(c p) d -> p c d", p=128)
                )
                nc.vector.memset(vh[:, :, D : D + 1], 1.0)

                # bf16 V working copy (cheap on DVE in 2x mode)
                vhb = heads.tile([128, NCHUNK, D + 1], bf16, tag="vhb")
                nc.vector.tensor_copy(out=vhb, in_=vh)

                # V_total colsum -> vtxb [1, D+1] bf16; element D = S exactly.
                # 8 accumulating ones^T matmuls fold partitions and chunk
                # groups; two small adds fold the remaining 4 groups.
                vt_ps = ps_s.tile([1, 4, D + 1], f32, tag="ps")
                for m in range(8):
                    nc.tensor.matmul(
                        vt_ps,
                        ones_colb,
                        vhb[:, 4 * m : 4 * (m + 1), :],
                        start=(m == 0),
                        stop=(m == 7),
                    )
                vt4 = vtp.tile([1, 4 * (D + 1)], f32, tag="vt4")
                nc.scalar.copy(out=vt4, in_=vt_ps.rearrange("p a b -> p (a b)"))
                vt2 = vtp.tile([1, 2 * (D + 1)], f32, tag="vt2")
                nc.vector.tensor_add(
                    vt2,
                    vt4[:, 0 : 2 * (D + 1)],
                    vt4[:, 2 * (D + 1) : 4 * (D + 1)],
                )
                vtxb = vtp.tile([1, D + 1], bf16, tag="vtxb")
                nc.vector.tensor_add(
                    vtxb, vt2[:, 0 : D + 1], vt2[:, D + 1 : 2 * (D + 1)]
                )

                for j in range(NPAIR):
                    c0 = 2 * j
                    # -- pair transposes: [128(cc*64+d), 128(s_local)] bf16 --
                    pt = ps_t.tile([128, 256], f32, tag="pt")
                    nc.tensor.transpose(
                        pt[:, 0:128],
                        qh[:, c0 : c0 + 2, :].rearrange("p a b -> p (a b)"),
                        ident,
                    )
                    nc.tensor.transpose(
                        pt[:, 128:256],
                        kh[:, c0 : c0 + 2, :].rearrange("p a b -> p (a b)"),
                        ident,
                    )
                    tsb = work.tile([128, 256], bf16, tag="tsb")
                    nc.vector.tensor_copy(out=tsb, in_=pt)

                    # -- scores S^T[k, q] per chunk -> one [128, 256] psum --
                    # NOTE: a regular matmul output must start at byte offset
                    # 0 of its PSUM bank (partition offsets are fine, column
                    # offsets fault on HW) -> one PSUM tile per chunk here.
                    ps0 = ps_s.tile([128, 128], f32, tag="ps", name=f"ps_{h}_{j}_0")
                    ps1 = ps_s.tile([128, 128], f32, tag="ps", name=f"ps_{h}_{j}_1")
                    pss = [ps0, ps1]
                    ets = []
                    for cc in range(2):
                        nc.tensor.matmul(
                            pss[cc],
                            tsb[64 * cc : 64 * (cc + 1), 128:256],
                            tsb[64 * cc : 64 * (cc + 1), 0:128],
                            start=True,
                            stop=True,
                        )
                        # E^T = exp(S^T/8); zero the cross-block quadrants
                        et = work.tile([128, 128], bf16, tag="et")
                        nc.scalar.activation(
                            out=et, in_=pss[cc], func=EXP, scale=SCALE
                        )
                        nc.gpsimd.memset(et[0:64, 64:128], 0.0)
                        nc.gpsimd.memset(et[64:128, 0:64], 0.0)
                        ets.append(et)

                    # -- num|denom per chunk --
                    po0 = ps_o.tile([128, D + 1], f32, tag="po", name=f"po_{h}_{j}_0")
                    po1 = ps_o.tile([128, D + 1], f32, tag="po", name=f"po_{h}_{j}_1")
                    pos = [po0, po1]
                    for cc in range(2):
                        c = c0 + cc
                        nc.tensor.matmul(
                            pos[cc], ets[cc], vhb[:, c, :], start=True, stop=False
                        )
                        nc.tensor.matmul(
                            pos[cc], negblk, vhb[:, c, :], start=False, stop=False
                        )
                        nc.tensor.matmul(
                            pos[cc], ones_row, vtxb, start=False, stop=True
                        )

                    # -- out = num * (1/denom) --
                    rcp = work.tile([128, 2], f32, tag="rcp")
                    for cc in range(2):
                        nc.vector.reciprocal(
                            out=rcp[:, cc : cc + 1], in_=pos[cc][:, D : D + 1]
                        )
                        nc.vector.tensor_scalar_mul(
                            oh[:, c0 + cc, :], pos[cc][:, 0:D], rcp[:, cc : cc + 1]
                        )

                nc.sync.dma_start(
                    out=o_d[h].rearrange("(c p) d -> p c d", p=128), in_=oh
                )

    nc.compile()
    return nc


def _get_compiled():
    if "nc" not in _CACHE:
        _CACHE["nc"] = _build_bass()
    return _CACHE["nc"]


def make_in_maps(query, key, value):
    q = np.ascontiguousarray(np.asarray(query).reshape(H, S, D), dtype=np.float32)
    k = np.ascontiguousarray(np.asarray(key).reshape(H, S, D), dtype=np.float32)
    v = np.ascontiguousarray(np.asarray(value).reshape(H, S, D), dtype=np.float32)
    in_maps = []
    for i in range(NCORES):
        sl = slice(i * HPC, (i + 1) * HPC)
        in_maps.append(
            {
                "query": np.ascontiguousarray(q[sl]),
                "key": np.ascontiguousarray(k[sl]),
                "value": np.ascontiguousarray(v[sl]),
            }
        )
    return in_maps


def run_spmd(in_maps, **kwargs):
    from concourse.bass_utils import run_bass_kernel_spmd

    nc = _get_compiled()
    return run_bass_kernel_spmd(nc, in_maps, core_ids=list(range(NCORES)), **kwargs)


def assemble(res):
    outs = [res.results[i]["out"] for i in range(NCORES)]
    return np.concatenate(outs, axis=0).reshape(1, H, S, D).astype(np.float32)


def kernel(query: np.ndarray, key: np.ndarray, value: np.ndarray) -> np.ndarray:
    return assemble(run_spmd(make_in_maps(query, key, value)))
